# revision 11
# baseline (speedup 1.0000x reference)
"""CQAttention (context-query attention, BiDAF/QANet-style) Trainium2 kernel.

Problem: B=8, Lc=2048, Lq=512, d=512.
  S[b,i,j] = C_i.wc + Q_j.wq + sum_k wm_k C_ik Q_jk + b  (trilinear score)
  Sq = softmax_j(S); Sc = softmax_i(S)
  A  = Sq @ Q;  Bm = Sq @ (Sc^T @ C)
  out = [C | A | C*A | C*Bm]   -> [B, Lc, 4d]

Strategy: data-parallel over batch across the 8 NeuronCores (one batch per
core).  Out block 0 (C verbatim) is assembled host-side, so the device only
computes/stores blocks 1-3 (A | C*A | C*Bm, [Lc, 3d]).

Per core (all matmul operands bf16, 1 cycle/row on the PE):

  P1: S = (C @ diag(wm) @ Q^T) + qb_j (K=1 aug matmul) ; per 128-row tile
  P2: En = exp(S + c_i)  (activation bias; accum_out -> rowsums)
  T : En -> Et via PE transpose (128x128 blocks into bf16 PSUM), then
      activation Copy PSUM->SBUF with accum_out -> colsum partials.
      This replaces the baseline's full recompute of the transposed score
      (saves ~13.7us of PE time and a full exp pass).
  P6: Abar = En @ Q   (lhsT = Et columns); A = Abar/rowsum, CA = C*A
  P5: F = En^T @ C    (lhsT = En columns); ScTC = F/colsum
  P7: Bmbar = En @ ScTC (lhsT = Et columns); CB = C*(Bmbar/rowsum)

Normalization is fused: A = psum*rsr on Pool (tensor_scalar_mul), CA/CB =
(psum*rsr)*Cbf in one scalar_tensor_tensor on DVE/Pool.  Bm itself is never
materialized.  Host side precomputes cheap O(L*d) vectors and transposes.
"""

import numpy as np

_B, _LC, _LQ, _D = 8, 2048, 512, 512
_P = 128


def _ensure_import():
    try:
        import concourse.bass  # noqa: F401
    except ImportError:
        import sys

        for p in ("/opt/trn_rl_repo", "/root/.axon_site/_ro/trn_rl_repo"):
            if p not in sys.path:
                sys.path.insert(0, p)
        import concourse.bass  # noqa: F401


def build_program(Lc=_LC, Lq=_LQ, D=_D):
    """Build the single-core Bass program (identical across the 8 cores).

    Scheduling notes (from HW traces of the predecessor kernel):
      - The PE clock is HAM-gated to 1.2 GHz until ~3.4us of sustained
        activity; K=1 warmup matmuls on the first-resident tiny tiles fill
        the DMA head so P1 starts at full clock.
      - PE emission order software-pipelines chunks so transposes never
        stall on exp and P6 never stalls on the Et copies:
        P1(c0) P1(c1) T(c0) P1(c2) P6(c0) T(c1) P1(c3) P6(c1) T(c2)
        P6(c2) T(c3) P6(c3) P5 P7.
    """
    _ensure_import()
    from contextlib import ExitStack

    import concourse.mybir as mybir
    from concourse import bacc
    from concourse.tile import TileContext

    f32 = mybir.dt.float32
    bf16 = mybir.dt.bfloat16
    EXP = mybir.ActivationFunctionType.Exp
    COPY = mybir.ActivationFunctionType.Copy
    MULT = mybir.AluOpType.mult
    AXX = mybir.AxisListType.X
    P = _P
    NLc, NLq, ND = Lc // P, Lq // P, D // P
    CHUNK = 512  # i-tile chunk: 4 row-tiles per pipeline stage
    NCH = Lc // CHUNK
    PCH = CHUNK // P

    nc = bacc.Bacc()
    dCT = nc.declare_dram_parameter("CT", [D, Lc], bf16, isOutput=False)
    dQmT = nc.declare_dram_parameter("QmT", [D, Lq], bf16, isOutput=False)
    dCbf = nc.declare_dram_parameter("Cbf", [Lc, D], bf16, isOutput=False)
    dQbf = nc.declare_dram_parameter("Qbf", [Lq, D], bf16, isOutput=False)
    dccols = nc.declare_dram_parameter("c_cols", [P, NLc], f32, isOutput=False)
    dqrow = nc.declare_dram_parameter("qb_row", [1, Lq], bf16, isOutput=False)
    dones = nc.declare_dram_parameter("ones_row", [1, P], bf16, isOutput=False)
    dident = nc.declare_dram_parameter("ident", [P, P], bf16, isOutput=False)
    dout = nc.declare_dram_parameter("out", [Lc, 3 * D], f32, isOutput=True)

    with ExitStack() as ctx:
        tc = ctx.enter_context(TileContext(nc))
        sb = ctx.enter_context(tc.tile_pool(name="persist", bufs=1))
        psum = ctx.enter_context(tc.tile_pool(name="psum", bufs=7, space="PSUM"))
        stage = ctx.enter_context(tc.tile_pool(name="stage", bufs=3))

        # ---- persistent SBUF tiles ----
        # wide tiles so each input streams in via ONE (or few) 3D-AP DMAs:
        # per-descriptor HWDGE submission costs ~625ns of SP time, and 44
        # per-tile submissions paced the whole input stream in the v1 trace.
        tCTall = sb.tile([P, ND * Lc], bf16, name="CTall")  # [n][k][c] chunk-major
        tCT = [
            [tCTall[:, (n * ND + k) * CHUNK : (n * ND + k + 1) * CHUNK] for n in range(NCH)]
            for k in range(ND)
        ]
        tQmTall = sb.tile([P, ND * Lq], bf16, name="QmTall")
        tQmT = [tQmTall[:, k * Lq : (k + 1) * Lq] for k in range(ND)]
        tCball = sb.tile([P, NLc * D], bf16, name="Cball")
        tCb = [tCball[:, i * D : (i + 1) * D] for i in range(NLc)]
        tQall = sb.tile([P, NLq * D], bf16, name="Qall")
        tQ = [tQall[:, j * D : (j + 1) * D] for j in range(NLq)]
        tEn = [sb.tile([P, Lq], bf16, tag=f"En{i}", name=f"En{i}") for i in range(NLc)]
        tEt = [sb.tile([P, Lc], bf16, tag=f"Et{j}", name=f"Et{j}") for j in range(NLq)]
        tSc = [sb.tile([P, D], bf16, tag=f"Sc{j}", name=f"Sc{j}") for j in range(NLq)]
        tI = sb.tile([P, P], bf16, name="ident")
        tcb = sb.tile([P, NLc], f32, name="cbias")
        tqrow = sb.tile([1, Lq], bf16, name="qrow")
        tones = sb.tile([1, P], bf16, name="ones")
        trsr = [sb.tile([P, 1], f32, tag=f"rsr{i}", name=f"rsr{i}") for i in range(NLc)]
        trs0 = [sb.tile([P, 1], f32, tag=f"rs0{i}", name=f"rs0{i}") for i in range(NLc)]
        tcsp = [
            sb.tile([P, NCH], f32, tag=f"csp{j}", name=f"csp{j}") for j in range(NLq)
        ]
        tcs0 = [sb.tile([P, 1], f32, tag=f"cs0{j}", name=f"cs0{j}") for j in range(NLq)]
        tcsr = [sb.tile([P, 1], f32, tag=f"csr{j}", name=f"csr{j}") for j in range(NLq)]

        # ---- input DMA (batched 3D APs, few submissions) ----
        # sync: warmup operands, then score operands in need-order
        nc.sync.dma_start(out=tones[:], in_=dones[:, :])
        nc.sync.dma_start(out=tqrow[:], in_=dqrow[:, :])
        for n in range(NCH):
            nc.sync.dma_start(
                out=tCTall[:, n * ND * CHUNK : (n + 1) * ND * CHUNK].rearrange(
                    "p (k c) -> p k c", k=ND
                ),
                in_=dCT[:, n * CHUNK : (n + 1) * CHUNK].rearrange(
                    "(k p) c -> p k c", k=ND
                ),
            )
            if n == 0:
                nc.sync.dma_start(
                    out=tQmTall[:, :].rearrange("p (k c) -> p k c", k=ND),
                    in_=dQmT[:, :].rearrange("(k p) c -> p k c", k=ND),
                )
                nc.sync.dma_start(
                    out=tQall[:, :].rearrange("p (j c) -> p j c", j=NLq),
                    in_=dQbf[:, :].rearrange("(j p) c -> p j c", j=NLq),
                )
        # scalar engine submits the rest in parallel (it is idle early)
        nc.scalar.dma_start(out=tcb[:], in_=dccols[:, :])
        nc.scalar.dma_start(out=tI[:], in_=dident[:, :])
        for n in range(NCH):
            nc.scalar.dma_start(
                out=tCball[:, n * PCH * D : (n + 1) * PCH * D].rearrange(
                    "p (i c) -> p i c", i=PCH
                ),
                in_=dCbf[n * CHUNK : (n + 1) * CHUNK, :].rearrange(
                    "(i p) c -> p i c", i=PCH
                ),
            )

        # ---- PE warmup: K=1 matmuls on the first-resident tiny tiles lift
        # the HAM clock-gate (needs ~3.4us sustained activity) during the
        # DMA head, so P1 starts at 2.4 GHz.
        warm_ps = psum.tile([P, Lq], f32, tag="warm", name="warm_ps", bufs=1)
        for _w in range(14):
            nc.tensor.matmul(warm_ps[:], tones[:], tqrow[:], start=True, stop=True)

        def p1_chunk(n):
            """Natural score + exp for the 4 row-tiles of chunk n."""
            for i in range(n * PCH, (n + 1) * PCH):
                ps = psum.tile([P, Lq], f32, tag="ps", name=f"psn{i}")
                for k in range(ND):
                    nc.tensor.matmul(
                        ps[:],
                        tCT[k][n][:, (i % PCH) * P : (i % PCH + 1) * P],
                        tQmT[k][:],
                        start=(k == 0),
                        stop=False,
                    )
                nc.tensor.matmul(ps[:], tones[:], tqrow[:], start=False, stop=True)
                nc.scalar.activation(
                    tEn[i][:], ps[:], EXP, bias=tcb[:, i : i + 1], accum_out=trs0[i][:]
                )
                nc.vector.reciprocal(trsr[i][:], trs0[i][:])

        def t_chunk(n):
            """PE-transpose En row-tiles of chunk n into Et columns."""
            sl = slice(n * CHUNK, (n + 1) * CHUNK)
            for j in range(NLq):
                psT = psum.tile([P, CHUNK], bf16, tag="ps", name=f"pst{n}_{j}")
                for ii in range(PCH):
                    i = n * PCH + ii
                    nc.tensor.transpose(
                        psT[:, ii * P : (ii + 1) * P],
                        tEn[i][:, j * P : (j + 1) * P],
                        tI[:],
                    )
                nc.scalar.activation(
                    tEt[j][:, sl], psT[:], COPY, accum_out=tcsp[j][:, n : n + 1]
                )

        def p6_chunk(n):
            """Abar = En @ Q for chunk n; A and C*A out (paired DMAs)."""
            for pair in range(PCH // 2):
                i0 = n * PCH + 2 * pair
                tACA = stage.tile([P, 4 * D], f32, tag="ACA", name=f"ACA{i0}")
                for q in range(2):
                    i = i0 + q
                    psA = psum.tile([P, D], f32, tag="ps", name=f"psa{i}")
                    for j in range(NLq):
                        nc.tensor.matmul(
                            psA[:],
                            tEt[j][:, i * P : (i + 1) * P],
                            tQ[j][:],
                            start=(j == 0),
                            stop=(j == NLq - 1),
                        )
                    tA = tACA[:, q * 2 * D : q * 2 * D + D]
                    nc.vector.tensor_scalar_mul(tA, psA[:], trsr[i][:])
                    nc.gpsimd.tensor_mul(
                        tACA[:, q * 2 * D + D : (q + 1) * 2 * D], tA, tCb[i][:]
                    )
                nc.sync.dma_start(
                    out=dout[i0 * P : (i0 + 2) * P, 0 : 2 * D].rearrange(
                        "(q p) c -> p q c", q=2
                    ),
                    in_=tACA[:, :].rearrange("p (q c) -> p q c", q=2),
                )

        # ---- software-pipelined chunk loop ----
        p1_chunk(0)
        p1_chunk(1)
        t_chunk(0)
        p1_chunk(2)
        p6_chunk(0)
        t_chunk(1)
        p1_chunk(3)
        p6_chunk(1)
        t_chunk(2)
        p6_chunk(2)
        t_chunk(3)
        p6_chunk(3)

        # ---- colsums ----
        for j in range(NLq):
            nc.vector.reduce_sum(tcs0[j][:], tcsp[j][:], axis=AXX)
            nc.vector.reciprocal(tcsr[j][:], tcs0[j][:])

        # ---- P5: F = En^T @ C -> ScTC ----
        for j in range(NLq):
            psF = psum.tile([P, D], f32, tag="ps", name=f"psf{j}")
            for k in range(NLc):
                nc.tensor.matmul(
                    psF[:],
                    tEn[k][:, j * P : (j + 1) * P],
                    tCb[k][:],
                    start=(k == 0),
                    stop=(k == NLc - 1),
                )
            nc.scalar.activation(tSc[j][:], psF[:], COPY, scale=tcsr[j][:])

        # ---- P7: Bmbar -> C*Bm out (paired DMAs) ----
        for pair in range(NLc // 2):
            i0 = 2 * pair
            tCB2 = stage.tile([P, 2 * D], f32, tag="CB", name=f"CB{i0}")
            for q in range(2):
                i = i0 + q
                psB = psum.tile([P, D], f32, tag="ps", name=f"psb{i}")
                for j in range(NLq):
                    nc.tensor.matmul(
                        psB[:],
                        tEt[j][:, i * P : (i + 1) * P],
                        tSc[j][:],
                        start=(j == 0),
                        stop=(j == NLq - 1),
                    )
                tBm = stage.tile([P, D], f32, tag="BM", name=f"Bm{i}")
                nc.vector.tensor_scalar_mul(tBm[:], psB[:], trsr[i][:])
                nc.gpsimd.tensor_mul(tCB2[:, q * D : (q + 1) * D], tBm[:], tCb[i][:])
            nc.sync.dma_start(
                out=dout[i0 * P : (i0 + 2) * P, 2 * D : 3 * D].rearrange(
                    "(q p) c -> p q c", q=2
                ),
                in_=tCB2[:, :].rearrange("p (q c) -> p q c", q=2),
            )

    nc.finalize()  # Bacc lowering: wait-splitting, reg alloc, nop fusion
    return nc


def prepare_in_maps(C, Q, Wo_w, Wo_b):
    """Shard over batch; per batch precompute layouts + rank-1 vectors."""
    import ml_dtypes

    bf16 = ml_dtypes.bfloat16
    D = C.shape[-1]
    w = np.asarray(Wo_w, np.float32)[0]
    wc, wq, wm = w[:D], w[D : 2 * D], w[2 * D :]
    b0 = np.float32(np.asarray(Wo_b, np.float32)[0])
    ones = np.ones((1, _P), bf16)
    ident = np.eye(_P, dtype=bf16)
    in_maps = []
    for b in range(C.shape[0]):
        Cb = np.ascontiguousarray(C[b], np.float32)
        Qb = np.ascontiguousarray(Q[b], np.float32)
        cvec = (Cb @ wc).astype(np.float32)
        qbvec = (Qb @ wq + b0).astype(np.float32)
        in_maps.append(
            {
                "CT": np.ascontiguousarray(Cb.T).astype(bf16),
                "QmT": np.ascontiguousarray((Qb * wm).T).astype(bf16),
                "Cbf": Cb.astype(bf16),
                "Qbf": Qb.astype(bf16),
                "c_cols": np.ascontiguousarray(cvec.reshape(-1, _P).T),
                "qb_row": qbvec[None, :].astype(bf16),
                "ones_row": ones,
                "ident": ident,
            }
        )
    return in_maps


_prog_cache = {}


def _get_program():
    if "nc" not in _prog_cache:
        _prog_cache["nc"] = build_program()
    return _prog_cache["nc"]


def run(C, Q, Wo_w, Wo_b, **spmd_kwargs):
    """Run on hardware; returns (out [B,Lc,4d], BassKernelResults)."""
    _ensure_import()
    from concourse.bass_utils import run_bass_kernel_spmd

    nc = _get_program()
    in_maps = prepare_in_maps(C, Q, Wo_w, Wo_b)
    res = run_bass_kernel_spmd(nc, in_maps, list(range(len(in_maps))), **spmd_kwargs)
    B, Lc, D = C.shape[0], C.shape[1], C.shape[2]
    out = np.empty((B, Lc, 4 * D), np.float32)
    out[:, :, :D] = C
    for i in range(B):
        out[i, :, D:] = res.results[i]["out"]
    return out, res


def kernel(C, Q, Wo_w, Wo_b):
    out, _ = run(C, Q, Wo_w, Wo_b)
    return out
